# revision 22
# baseline (speedup 1.0000x reference)
"""Trainium2 Bass kernel for windowed multi-lag autocorrelation.

Reference computation (per (batch, seq) row of x[16, 128, 8320]):
  - 64 overlapping windows of length 256, stride 128
  - per-window mean removal, hanning window
  - autocorrelation at lags 0..31, scaled by 1/256
  -> out [16, 128, 1, 64, 32]

Device formulation (quadratic op -> DFT trick so the PE does the work):
  autocorr(w)[a] = (1/N) sum_f alpha_f |DFT_N(w)|^2[f] * cos(2*pi*f*a/N)
  with N = 256 (= WIN_LEN, NOT >= WIN_LEN+31): the circular-vs-linear wrap
  terms pair the first a samples with the last a samples of the window,
  both crushed by the hanning taper -- measured wrap error 7e-5 l2, far
  under the 2e-2 gate. N=256 makes the forward transform exactly 256
  outputs = 2 row-blocks of 128 (cos 0..127 | cos128, sin 1..127), so the
  forward is 4 matmuls per group (2 blocks x 2 window-halves) instead of
  6 for N=288, and the inverse needs no cos^2+sin^2 pairing pass at all:
  2 matmuls on the squared blocks directly (B row 0 of block 2 carries
  bin 128's (-1)^a weights).

  Everything runs in float16 (11-bit significand ~ fp32r's 12): fp16
  streams the PE at 1 col/cycle like bf16 AND gets fast weight loads --
  the fp32r baseline's matmuls ran in fp32_mode=HIGH at ~390ns each vs
  ~216ns for 16-bit. Mean removal + hanning fold into the forward matrix;
  a 1/16 scale on A keeps |X| and B in fp16 normal range.

  The two inverse matmuls have only 32 output rows each, so they are
  column-tiled ((0,0) and (0,32), auto-derived from the PSUM slice base
  partition) and execute concurrently in different column strips of the
  PE array: ~216ns for the pair. Their 32-row halves land in partitions
  0:32 / 32:64 and are summed on the host (cross-partition adds are not
  a DVE thing).

  Per-group engine budget (warm): PE 4x216+216 ~ 1.1us; ScalarE squares
  the whole [128, 1024] PSUM block in one ACTIVATE (~1.15us -- a DVE
  square is illegal, TensorTensor may read only one input from PSUM);
  VectorE copies the previous group's [64, 512] inverse result out.

Sharding: pure data parallel, 2 batches per core across 8 cores.
"""
import os
import numpy as np

import concourse.bass as bass
import concourse.tile as tile
from concourse import mybir
from concourse.bass_utils import run_bass_kernel_spmd

NUM_AUTOCORR = 32
NUM_WINDOWS = 64
WIN_LEN = 256
WIN_STRIDE = 128
NFFT = 256
SEQ = 128
BATCH = 16
VALUE = (NUM_WINDOWS - 1) * WIN_STRIDE + WIN_LEN  # 8320
NCHUNK = VALUE // WIN_STRIDE  # 65
N_CORES = 8
ROWS_PER_CORE = (BATCH // N_CORES) * SEQ  # 256
G = 8  # rows per group
NGROUP = ROWS_PER_CORE // G  # 32
NW = G * NUM_WINDOWS  # 512 windows per group (matmul free dim)
GW = G * NCHUNK  # 520 columns per group in the input tile
SA = 1.0 / 16.0  # forward-matrix scale keeping fp16 ranges comfortable
ACOL = 4 * 128  # A as 4 [128,128] lhsT tiles: (half q, block b) at (2q+b)*128
BCOL = 2 * NUM_AUTOCORR  # B1 | B2
CCOL = ACOL + BCOL  # 576 const columns
# progressive input DMA chunking: small first chunks so the PE starts early,
# big later chunks so descriptors stay large. The constants ride in their
# own tiny first DMA so weight loads can begin before group 0's data lands.
IN_CHUNKS = [1, 1, 2, 4, 8, 8, 8]
assert sum(IN_CHUNKS) == NGROUP

F32 = mybir.dt.float32
F16 = mybir.dt.float16

LAST_EXEC_NS = None


def _build_mats():
    i = np.arange(WIN_LEN)
    h = np.hanning(WIN_LEN)
    f = np.arange(NFFT // 2 + 1)  # 0..128
    ang = 2 * np.pi * np.outer(i, f) / NFFT
    C = h[:, None] * np.cos(ang)
    S = h[:, None] * np.sin(ang)
    # column layout: block 0 = cos 0..127; block 1 = [cos 128, sin 1..127]
    A = np.empty((WIN_LEN, 2 * 128), np.float64)
    A[:, 0:128] = C[:, 0:128]
    A[:, 128] = C[:, 128]
    A[:, 129:256] = S[:, 1:128]
    A = A - A.mean(axis=0, keepdims=True)  # fold per-window mean removal
    A *= SA
    a = np.arange(NUM_AUTOCORR)
    fa = 2 * np.pi * np.outer(np.arange(128), a) / NFFT
    alpha = np.full(128, 2.0)
    alpha[0] = 1.0
    B = np.empty((128, BCOL), np.float64)
    B[:, 0:32] = alpha[:, None] * np.cos(fa) / NFFT  # vs cos^2 block
    B[:, 32:64] = B[:, 0:32]  # sin^2 rows 1..127 share the cos weights
    B[0, 32:64] = np.cos(np.pi * a) / NFFT  # row 0 of block 2 is bin 128
    return A.astype(np.float16), B.astype(np.float16)


def _split_sync_waits(nc, max_waits=1):
    """walrus in this container rejects instructions with multiple sem waits
    ("Too many sync wait commands"); split extras into single-wait NoOps."""
    ctr = [0]

    def mknop(engine, waits):
        ctr[0] += 1
        nop = mybir.InstNoOp(name=f"waitsplit-{ctr[0]}", ins=[], outs=[])
        nop.engine = engine
        nop.sync_info = mybir.SyncInfo(on_wait=list(waits), on_update=[])
        return nop

    for fn in nc.m.functions:
        for blk in fn.blocks:
            out = []
            changed = False
            for inst in blk.instructions:
                si = inst.sync_info
                waits = list(si.on_wait) if si is not None and si.on_wait else []
                if len(waits) > max_waits:
                    changed = True
                    extra, keep = waits[:-max_waits], waits[-max_waits:]
                    for k in range(0, len(extra), max_waits):
                        out.append(mknop(inst.engine, extra[k : k + max_waits]))
                    inst.sync_info = mybir.SyncInfo(
                        on_wait=keep, on_update=list(si.on_update or [])
                    )
                out.append(inst)
            if changed:
                blk.instructions = out
    return nc


def _build_kernel():
    nc = bass.Bass(target_bir_lowering=False)
    # p-major layout: xt[p, CCOL consts then ((g r) c)] with
    # xt[p, CCOL + (8g+r)*65 + c] = x[row 8g+r, 128c + p]; any column-range
    # DMA slice is per-partition contiguous in DRAM
    xt = nc.dram_tensor("xt", [128, CCOL + NGROUP * GW], F16, kind="ExternalInput")
    out = nc.dram_tensor(
        "out", [NGROUP // 2, 4 * NUM_AUTOCORR, NW], F32, kind="ExternalOutput"
    )

    with tile.TileContext(nc) as tc:
        with (
            tc.tile_pool(name="xin", bufs=1) as xpool,
            tc.tile_pool(name="mid", bufs=6) as mpool,
            tc.tile_pool(name="outb", bufs=8) as opool,
            tc.tile_pool(name="ps", bufs=2, space="PSUM") as pspool,
            tc.tile_pool(name="pso", bufs=4, space="PSUM") as psopool,
        ):
            c0 = xpool.tile([128, CCOL], F16, tag="consts")
            nc.sync.dma_start(c0[:], xt.ap()[:, 0:CCOL])

            chunk_tiles = []  # (tile, first_group, n_groups)
            g0 = 0
            for ci, sz in enumerate(IN_CHUNKS):
                cols = sz * GW
                xc_t = xpool.tile([128, cols], F16, tag=f"xc{ci}")
                lo = CCOL + g0 * GW
                nc.sync.dma_start(xc_t[:], xt.ap()[:, lo : lo + cols])
                chunk_tiles.append((xc_t, g0, sz))
                g0 += sz

            # a_r[q][b]: lhsT for window-half q, frequency block b
            a_r = [
                [c0[:, (2 * q + b) * 128 : (2 * q + b + 1) * 128] for b in range(2)]
                for q in range(2)
            ]
            b1_r = c0[:, ACOL : ACOL + NUM_AUTOCORR]
            b2_r = c0[:, ACOL + NUM_AUTOCORR : CCOL]

            def group_view(g):
                for t, gg0, sz in chunk_tiles:
                    if gg0 <= g < gg0 + sz:
                        lo = (g - gg0) * GW
                        return t[:, lo : lo + GW].rearrange("p (r c) -> p r c", r=G)
                raise AssertionError

            def flush_pair(ga, sqa, gb, sqb):
                # four column-tiled M=32 matmuls, concurrent in the PE array
                # (strips q0/q32/q64/q96 via the PSUM slice base partition):
                # both groups' inverses stream together in ~one matmul time.
                # Partition layout: [ga*B1 | ga*B2 | gb*B1 | gb*B2]
                ps_out = psopool.tile([4 * NUM_AUTOCORR, NW], F32, tag="ps_out")
                for j, (b_r, rhs) in enumerate(
                    (
                        (b1_r, sqa[0][:]),
                        (b2_r, sqa[1][:]),
                        (b1_r, sqb[0][:]),
                        (b2_r, sqb[1][:]),
                    )
                ):
                    nc.tensor.matmul(
                        ps_out[j * NUM_AUTOCORR : (j + 1) * NUM_AUTOCORR, :],
                        b_r, rhs, start=True, stop=True,
                        tile_position=(0, j * NUM_AUTOCORR),
                    )
                o_sb = opool.tile([4 * NUM_AUTOCORR, NW], F32, tag="o_sb")
                nc.vector.tensor_copy(o_sb[:], ps_out[:])
                nc.sync.dma_start(out.ap()[ga // 2], o_sb[:])

            # inverses lag their forwards by 2-3 groups and are flushed in
            # pairs: the square's ACTIVATE latency is fully off the critical
            # path, and the LDWEIGHTS bubble at the inverse->forward weight
            # switch is paid once per TWO groups
            pend = []  # [(g, sq), ...]
            for g in range(NGROUP):
                xv = group_view(g)

                # forward DFT: 2 frequency blocks x 2 window-halves, each
                # block PSUM-accumulates its two halves
                ps_all = pspool.tile([128, 2 * NW], F32, tag="ps_all")
                for b in (0, 1):
                    nc.tensor.matmul(
                        ps_all[:, b * NW : (b + 1) * NW], a_r[0][b],
                        xv[:, :, 0:NUM_WINDOWS], start=True, stop=False,
                    )
                    nc.tensor.matmul(
                        ps_all[:, b * NW : (b + 1) * NW], a_r[1][b],
                        xv[:, :, 1 : NUM_WINDOWS + 1], start=False, stop=True,
                    )

                if g % 2 == 0 and len(pend) >= 3:
                    (ga, sqa), (gb, sqb) = pend[0], pend[1]
                    pend = pend[2:]
                    flush_pair(ga, sqa, gb, sqb)

                # power spectrum, spread over three engines so each PSUM
                # bank releases ~700ns after its forward (a single 1024-col
                # ScalarE ACTIVATE saturates ScalarE and holds both banks):
                # ScalarE squares block 0 from PSUM; VectorE downcasts block
                # 1 to fp16 in SBUF; GpSimd (idle otherwise, no PSUM port)
                # squares the SBUF copy. fp16(X)^2 == fp16(X^2) error-wise.
                sq0 = mpool.tile([128, NW], F16, tag="sq0")
                nc.scalar.square(sq0[:], ps_all[:, 0:NW])
                t2 = mpool.tile([128, NW], F16, tag="t2")
                nc.vector.tensor_copy(t2[:], ps_all[:, NW : 2 * NW])
                sq1 = mpool.tile([128, NW], F16, tag="sq1")
                nc.gpsimd.tensor_mul(sq1[:], t2[:], t2[:])
                pend.append((g, (sq0, sq1)))

            while pend:
                (ga, sqa), (gb, sqb) = pend[0], pend[1]
                pend = pend[2:]
                flush_pair(ga, sqa, gb, sqb)

    _split_sync_waits(nc)
    return nc


def _install_ntff_shim():
    """The trimmed antenv lacks axon_hooks, so trace=True degrades to no
    profile. Recreate the hook: ctypes into libaxon_pjrt.so (same ABI the
    boot shim uses), exposed as a synthetic antenv.axon_hooks module."""
    import sys
    import ctypes
    import contextlib
    import types

    if "antenv.axon_hooks" in sys.modules:
        return
    so_path = "/opt/axon/libaxon_pjrt.so"
    if not os.path.exists(so_path):
        return
    lib = ctypes.CDLL(so_path)
    if not hasattr(lib, "axon_start_nrt_profile"):
        return
    lib.axon_start_nrt_profile.argtypes = [
        ctypes.POINTER(ctypes.c_int64),
        ctypes.c_size_t,
    ]
    lib.axon_start_nrt_profile.restype = ctypes.c_int64
    lib.axon_stop_nrt_profile.argtypes = [ctypes.c_char_p]
    lib.axon_stop_nrt_profile.restype = ctypes.c_int64

    @contextlib.contextmanager
    def _hook(output_dir, device_ids):
        import jax

        jax.devices()
        if device_ids:
            ids = (ctypes.c_int64 * len(device_ids))(*device_ids)
            rc = lib.axon_start_nrt_profile(ids, len(device_ids))
        else:
            rc = lib.axon_start_nrt_profile(None, 0)
        if rc != 0:
            raise RuntimeError(f"axon_start_nrt_profile rc={rc}")
        try:
            yield
        finally:
            n = lib.axon_stop_nrt_profile(str(output_dir).encode())
            print(f"ntff profile: {n} file(s) -> {output_dir}")

    mod = types.ModuleType("antenv.axon_hooks")
    mod.get_axon_ntff_profile_hook = lambda: _hook
    mod.set_axon_ntff_profile_hook = lambda h: None
    sys.modules["antenv.axon_hooks"] = mod

    # avoid network-dependent artifact uploads in the trace path
    import concourse.bass_utils as bu

    bu.upload_artifacts = lambda tmpdir: f"local://{tmpdir}"


_NC_CACHE = None


def _get_nc():
    global _NC_CACHE
    if _NC_CACHE is None:
        _NC_CACHE = _build_kernel()
    return _NC_CACHE


def kernel(x: np.ndarray) -> np.ndarray:
    global LAST_EXEC_NS
    x = np.asarray(x)
    assert x.shape == (BATCH, SEQ, VALUE)

    A, B = _build_mats()
    consts = np.zeros((128, CCOL), np.float16)
    # A as 4 [128, 128] lhsT tiles: consts[p, (2q+b)*128 + m] = A[q*128+p, b*128+m]
    for q in range(2):
        for b in range(2):
            consts[:, (2 * q + b) * 128 : (2 * q + b + 1) * 128] = A[
                q * 128 : (q + 1) * 128, b * 128 : (b + 1) * 128
            ]
    consts[:, ACOL:CCOL] = B

    x16 = np.ascontiguousarray(x, dtype=np.float16)
    bpc = BATCH // N_CORES
    in_maps = []
    for c in range(N_CORES):
        xc = x16[c * bpc : (c + 1) * bpc]  # [2, 128, 8320]
        # xt[p, ((g r) c)] = x[row 8g+r, 128c + p], consts prepended
        xd = (
            xc.reshape(ROWS_PER_CORE, NCHUNK, WIN_STRIDE)  # [row, c, p]
            .transpose(2, 0, 1)  # [128 p, 256 row, 65 c]
            .reshape(128, NGROUP * GW)
        )
        xt = np.concatenate([consts, xd], axis=1)
        in_maps.append({"xt": xt})

    nc = _get_nc()
    trace = os.environ.get("AUTOCORR_TRACE", "0") == "1"
    if trace:
        _install_ntff_shim()
    res = run_bass_kernel_spmd(nc, in_maps, core_ids=list(range(N_CORES)), trace=trace)
    LAST_EXEC_NS = res.exec_time_ns

    outs = []
    for c in range(N_CORES):
        o = res.results[c]["out"]  # [NGROUP//2, 128, NW]; rows = (gpair half, B-block)
        o = o.reshape(NGROUP // 2, 2, 2, NUM_AUTOCORR, G, NUM_WINDOWS)
        o = o.sum(axis=2)  # sum the B1/B2 column-tile halves -> [pair, gAB, a, r, w]
        o = o.transpose(0, 1, 3, 4, 2)  # [pair, gAB, r, w, a]
        outs.append(o.reshape(bpc, SEQ, NUM_WINDOWS, NUM_AUTOCORR))
    full = np.concatenate(outs, axis=0)  # [16, 128, 64, 32]
    return np.ascontiguousarray(full[:, :, None, :, :].astype(np.float32))


# revision 25
# speedup vs baseline: 1.3389x; 1.3389x over previous
"""Trainium2 Bass kernel for windowed multi-lag autocorrelation.

Reference computation (per (batch, seq) row of x[16, 128, 8320]):
  - 64 overlapping windows of length 256, stride 128
  - per-window mean removal, hanning window
  - autocorrelation at lags 0..31, scaled by 1/256
  -> out [16, 128, 1, 64, 32]

Device formulation (quadratic op -> DFT trick so the PE does the work):
  autocorr(w)[a] = (1/N) sum_f alpha_f |DFT_N(w)|^2[f] * cos(2*pi*f*a/N)
  with N = 256 (= WIN_LEN, NOT >= WIN_LEN+31): the circular-vs-linear wrap
  terms pair the first a samples with the last a samples of the window,
  both crushed by the hanning taper -- measured wrap error 7e-5 l2, far
  under the 2e-2 gate. N=256 makes the forward transform exactly 256
  outputs = 2 row-blocks of 128 (cos 0..127 | cos128, sin 1..127), so the
  forward is 4 matmuls per group (2 blocks x 2 window-halves) instead of
  6 for N=288, and the inverse needs no cos^2+sin^2 pairing pass at all:
  2 matmuls on the squared blocks directly (B row 0 of block 2 carries
  bin 128's (-1)^a weights).

  Everything runs in float16 (11-bit significand ~ fp32r's 12): fp16
  streams the PE at 1 col/cycle like bf16 AND gets fast weight loads --
  the fp32r baseline's matmuls ran in fp32_mode=HIGH at ~390ns each vs
  ~216ns for 16-bit. Mean removal + hanning fold into the forward matrix;
  a 1/16 scale on A keeps |X| and B in fp16 normal range.

  The two inverse matmuls have only 32 output rows each, so they are
  column-tiled ((0,0) and (0,32), auto-derived from the PSUM slice base
  partition) and execute concurrently in different column strips of the
  PE array: ~216ns for the pair. Their 32-row halves land in partitions
  0:32 / 32:64 and are summed on the host (cross-partition adds are not
  a DVE thing).

  Per-group engine budget (warm): PE 4x216+216 ~ 1.1us; ScalarE squares
  the whole [128, 1024] PSUM block in one ACTIVATE (~1.15us -- a DVE
  square is illegal, TensorTensor may read only one input from PSUM);
  VectorE copies the previous group's [64, 512] inverse result out.

Sharding: pure data parallel, 2 batches per core across 8 cores.
"""
import os
import numpy as np

import concourse.bass as bass
import concourse.tile as tile
from concourse import mybir
from concourse.bass_utils import run_bass_kernel_spmd

NUM_AUTOCORR = 32
NUM_WINDOWS = 64
WIN_LEN = 256
WIN_STRIDE = 128
NFFT = 256
SEQ = 128
BATCH = 16
VALUE = (NUM_WINDOWS - 1) * WIN_STRIDE + WIN_LEN  # 8320
NCHUNK = VALUE // WIN_STRIDE  # 65
N_CORES = 8
ROWS_PER_CORE = (BATCH // N_CORES) * SEQ  # 256
G = 8  # rows per group
NGROUP = ROWS_PER_CORE // G  # 32
NW = G * NUM_WINDOWS  # 512 windows per group (matmul free dim)
GW = G * NCHUNK  # 520 columns per group in the input tile
SA = 1.0 / 16.0  # forward-matrix scale keeping fp16 ranges comfortable
ACOL = 4 * 128  # A as 4 [128,128] lhsT tiles: (half q, block b) at (2q+b)*128
BCOL = 2 * NUM_AUTOCORR  # B1 | B2
CCOL = ACOL + BCOL  # 576 const columns
# progressive input DMA chunking: small first chunks so the PE starts early,
# big later chunks so descriptors stay large. The constants ride in their
# own tiny first DMA so weight loads can begin before group 0's data lands.
IN_CHUNKS = [1, 1, 2, 4, 8, 8, 8]
assert sum(IN_CHUNKS) == NGROUP

F32 = mybir.dt.float32
F16 = mybir.dt.float16

LAST_EXEC_NS = None


def _build_mats():
    i = np.arange(WIN_LEN)
    h = np.hanning(WIN_LEN)
    f = np.arange(NFFT // 2 + 1)  # 0..128
    ang = 2 * np.pi * np.outer(i, f) / NFFT
    C = h[:, None] * np.cos(ang)
    S = h[:, None] * np.sin(ang)
    # column layout: block 0 = cos 0..127; block 1 = [cos 128, sin 1..127]
    A = np.empty((WIN_LEN, 2 * 128), np.float64)
    A[:, 0:128] = C[:, 0:128]
    A[:, 128] = C[:, 128]
    A[:, 129:256] = S[:, 1:128]
    A = A - A.mean(axis=0, keepdims=True)  # fold per-window mean removal
    A *= SA
    a = np.arange(NUM_AUTOCORR)
    fa = 2 * np.pi * np.outer(np.arange(128), a) / NFFT
    alpha = np.full(128, 2.0)
    alpha[0] = 1.0
    B = np.empty((128, BCOL), np.float64)
    B[:, 0:32] = alpha[:, None] * np.cos(fa) / NFFT  # vs cos^2 block
    B[:, 32:64] = B[:, 0:32]  # sin^2 rows 1..127 share the cos weights
    B[0, 32:64] = np.cos(np.pi * a) / NFFT  # row 0 of block 2 is bin 128
    return A.astype(np.float16), B.astype(np.float16)


def _split_sync_waits(nc, max_waits=1):
    """walrus in this container rejects instructions with multiple sem waits
    ("Too many sync wait commands"); split extras into single-wait NoOps."""
    ctr = [0]

    def mknop(engine, waits):
        ctr[0] += 1
        nop = mybir.InstNoOp(name=f"waitsplit-{ctr[0]}", ins=[], outs=[])
        nop.engine = engine
        nop.sync_info = mybir.SyncInfo(on_wait=list(waits), on_update=[])
        return nop

    for fn in nc.m.functions:
        for blk in fn.blocks:
            out = []
            changed = False
            for inst in blk.instructions:
                si = inst.sync_info
                waits = list(si.on_wait) if si is not None and si.on_wait else []
                if len(waits) > max_waits:
                    changed = True
                    extra, keep = waits[:-max_waits], waits[-max_waits:]
                    for k in range(0, len(extra), max_waits):
                        out.append(mknop(inst.engine, extra[k : k + max_waits]))
                    inst.sync_info = mybir.SyncInfo(
                        on_wait=keep, on_update=list(si.on_update or [])
                    )
                out.append(inst)
            if changed:
                blk.instructions = out
    return nc


def _build_kernel():
    nc = bass.Bass(target_bir_lowering=False)
    # p-major layout: xt[p, CCOL consts then ((g r) c)] with
    # xt[p, CCOL + (8g+r)*65 + c] = x[row 8g+r, 128c + p]; any column-range
    # DMA slice is per-partition contiguous in DRAM
    xt = nc.dram_tensor("xt", [128, CCOL + NGROUP * GW], F16, kind="ExternalInput")
    out = nc.dram_tensor(
        "out", [NGROUP // 2, 4 * NUM_AUTOCORR, NW], F32, kind="ExternalOutput"
    )

    with tile.TileContext(nc) as tc:
        with (
            tc.tile_pool(name="xin", bufs=1) as xpool,
            tc.tile_pool(name="mid", bufs=6) as mpool,
            tc.tile_pool(name="outb", bufs=8) as opool,
            tc.tile_pool(name="ps", bufs=2, space="PSUM") as pspool,
            tc.tile_pool(name="pso", bufs=4, space="PSUM") as psopool,
        ):
            # consts + first data chunk go out on the ScalarE HWDGE ring so
            # their issue doesn't queue behind the Sync ring's later chunks
            c0 = xpool.tile([128, CCOL], F16, tag="consts")
            nc.scalar.dma_start(c0[:], xt.ap()[:, 0:CCOL])

            chunk_tiles = []  # (tile, first_group, n_groups)
            g0 = 0
            for ci, sz in enumerate(IN_CHUNKS):
                cols = sz * GW
                xc_t = xpool.tile([128, cols], F16, tag=f"xc{ci}")
                lo = CCOL + g0 * GW
                eng = nc.scalar if ci == 0 else nc.sync
                eng.dma_start(xc_t[:], xt.ap()[:, lo : lo + cols])
                chunk_tiles.append((xc_t, g0, sz))
                g0 += sz

            # a_r[q][b]: lhsT for window-half q, frequency block b
            a_r = [
                [c0[:, (2 * q + b) * 128 : (2 * q + b + 1) * 128] for b in range(2)]
                for q in range(2)
            ]
            b1_r = c0[:, ACOL : ACOL + NUM_AUTOCORR]
            b2_r = c0[:, ACOL + NUM_AUTOCORR : CCOL]

            def group_view(g):
                for t, gg0, sz in chunk_tiles:
                    if gg0 <= g < gg0 + sz:
                        lo = (g - gg0) * GW
                        return t[:, lo : lo + GW].rearrange("p (r c) -> p r c", r=G)
                raise AssertionError

            def flush_pair(ga, sqa, gb, sqb):
                # four column-tiled M=32 matmuls, concurrent in the PE array
                # (strips q0/q32/q64/q96 via the PSUM slice base partition):
                # both groups' inverses stream together in ~one matmul time.
                # Partition layout: [ga*B1 | ga*B2 | gb*B1 | gb*B2]
                ps_out = psopool.tile([4 * NUM_AUTOCORR, NW], F32, tag="ps_out")
                for j, (b_r, rhs) in enumerate(
                    (
                        (b1_r, sqa[0]),
                        (b2_r, sqa[1]),
                        (b1_r, sqb[0]),
                        (b2_r, sqb[1]),
                    )
                ):
                    nc.tensor.matmul(
                        ps_out[j * NUM_AUTOCORR : (j + 1) * NUM_AUTOCORR, :],
                        b_r, rhs, start=True, stop=True,
                        tile_position=(0, j * NUM_AUTOCORR),
                    )
                o_sb = opool.tile([4 * NUM_AUTOCORR, NW], F32, tag="o_sb")
                nc.vector.tensor_copy(o_sb[:], ps_out[:])
                nc.sync.dma_start(out.ap()[ga // 2], o_sb[:])

            # inverses lag their forwards by 2-3 groups and are flushed in
            # pairs: the square's ACTIVATE latency is fully off the critical
            # path, and the LDWEIGHTS bubble at the inverse->forward weight
            # switch is paid once per TWO groups
            pend = []  # [(g, sq), ...]
            for g in range(NGROUP):
                xv = group_view(g)

                # forward DFT: 2 frequency blocks x 2 window-halves, each
                # block PSUM-accumulates its two halves
                ps_all = pspool.tile([128, 2 * NW], F32, tag="ps_all")
                for b in (0, 1):
                    nc.tensor.matmul(
                        ps_all[:, b * NW : (b + 1) * NW], a_r[0][b],
                        xv[:, :, 0:NUM_WINDOWS], start=True, stop=False,
                    )
                    nc.tensor.matmul(
                        ps_all[:, b * NW : (b + 1) * NW], a_r[1][b],
                        xv[:, :, 1 : NUM_WINDOWS + 1], start=False, stop=True,
                    )

                if g % 2 == 0 and len(pend) >= 3:
                    (ga, sqa), (gb, sqb) = pend[0], pend[1]
                    pend = pend[2:]
                    flush_pair(ga, sqa, gb, sqb)

                # power spectrum: one fused square on ScalarE (it has its
                # own PSUM port; VectorE only does the output copies.
                # GpSimd TENSOR_TENSOR measured ~2ns/col -- too slow to help)
                sq = mpool.tile([128, 2 * NW], F16, tag="sq")
                nc.scalar.square(sq[:], ps_all[:])
                pend.append((g, (sq[:, 0:NW], sq[:, NW : 2 * NW])))

            while pend:
                (ga, sqa), (gb, sqb) = pend[0], pend[1]
                pend = pend[2:]
                flush_pair(ga, sqa, gb, sqb)

    _split_sync_waits(nc)
    return nc


def _install_ntff_shim():
    """The trimmed antenv lacks axon_hooks, so trace=True degrades to no
    profile. Recreate the hook: ctypes into libaxon_pjrt.so (same ABI the
    boot shim uses), exposed as a synthetic antenv.axon_hooks module."""
    import sys
    import ctypes
    import contextlib
    import types

    if "antenv.axon_hooks" in sys.modules:
        return
    so_path = "/opt/axon/libaxon_pjrt.so"
    if not os.path.exists(so_path):
        return
    lib = ctypes.CDLL(so_path)
    if not hasattr(lib, "axon_start_nrt_profile"):
        return
    lib.axon_start_nrt_profile.argtypes = [
        ctypes.POINTER(ctypes.c_int64),
        ctypes.c_size_t,
    ]
    lib.axon_start_nrt_profile.restype = ctypes.c_int64
    lib.axon_stop_nrt_profile.argtypes = [ctypes.c_char_p]
    lib.axon_stop_nrt_profile.restype = ctypes.c_int64

    @contextlib.contextmanager
    def _hook(output_dir, device_ids):
        import jax

        jax.devices()
        if device_ids:
            ids = (ctypes.c_int64 * len(device_ids))(*device_ids)
            rc = lib.axon_start_nrt_profile(ids, len(device_ids))
        else:
            rc = lib.axon_start_nrt_profile(None, 0)
        if rc != 0:
            raise RuntimeError(f"axon_start_nrt_profile rc={rc}")
        try:
            yield
        finally:
            n = lib.axon_stop_nrt_profile(str(output_dir).encode())
            print(f"ntff profile: {n} file(s) -> {output_dir}")

    mod = types.ModuleType("antenv.axon_hooks")
    mod.get_axon_ntff_profile_hook = lambda: _hook
    mod.set_axon_ntff_profile_hook = lambda h: None
    sys.modules["antenv.axon_hooks"] = mod

    # avoid network-dependent artifact uploads in the trace path
    import concourse.bass_utils as bu

    bu.upload_artifacts = lambda tmpdir: f"local://{tmpdir}"


_NC_CACHE = None


def _get_nc():
    global _NC_CACHE
    if _NC_CACHE is None:
        _NC_CACHE = _build_kernel()
    return _NC_CACHE


def kernel(x: np.ndarray) -> np.ndarray:
    global LAST_EXEC_NS
    x = np.asarray(x)
    assert x.shape == (BATCH, SEQ, VALUE)

    A, B = _build_mats()
    consts = np.zeros((128, CCOL), np.float16)
    # A as 4 [128, 128] lhsT tiles: consts[p, (2q+b)*128 + m] = A[q*128+p, b*128+m]
    for q in range(2):
        for b in range(2):
            consts[:, (2 * q + b) * 128 : (2 * q + b + 1) * 128] = A[
                q * 128 : (q + 1) * 128, b * 128 : (b + 1) * 128
            ]
    consts[:, ACOL:CCOL] = B

    x16 = np.ascontiguousarray(x, dtype=np.float16)
    bpc = BATCH // N_CORES
    in_maps = []
    for c in range(N_CORES):
        xc = x16[c * bpc : (c + 1) * bpc]  # [2, 128, 8320]
        # xt[p, ((g r) c)] = x[row 8g+r, 128c + p], consts prepended
        xd = (
            xc.reshape(ROWS_PER_CORE, NCHUNK, WIN_STRIDE)  # [row, c, p]
            .transpose(2, 0, 1)  # [128 p, 256 row, 65 c]
            .reshape(128, NGROUP * GW)
        )
        xt = np.concatenate([consts, xd], axis=1)
        in_maps.append({"xt": xt})

    nc = _get_nc()
    trace = os.environ.get("AUTOCORR_TRACE", "0") == "1"
    if trace:
        _install_ntff_shim()
    res = run_bass_kernel_spmd(nc, in_maps, core_ids=list(range(N_CORES)), trace=trace)
    LAST_EXEC_NS = res.exec_time_ns

    outs = []
    for c in range(N_CORES):
        o = res.results[c]["out"]  # [NGROUP//2, 128, NW]; rows = (gpair half, B-block)
        o = o.reshape(NGROUP // 2, 2, 2, NUM_AUTOCORR, G, NUM_WINDOWS)
        o = o.sum(axis=2)  # sum the B1/B2 column-tile halves -> [pair, gAB, a, r, w]
        o = o.transpose(0, 1, 3, 4, 2)  # [pair, gAB, r, w, a]
        outs.append(o.reshape(bpc, SEQ, NUM_WINDOWS, NUM_AUTOCORR))
    full = np.concatenate(outs, axis=0)  # [16, 128, 64, 32]
    return np.ascontiguousarray(full[:, :, None, :, :].astype(np.float32))
